# revision 34
# baseline (speedup 1.0000x reference)
"""Trainium2 Bass kernel for nn_EnergyAE (B=64, D=12288, N=32, H=2048) on 8 cores.

v4: fp8 DoubleRow matmuls + 2 merged banded fp8 AllReduces + bf16 LDLT chain.
  phase E: encoder (bf16) -> z* partial -> AllGather+sum (8KB)
  phase C: C = W2 @ W2^T upper row-band rectangles in fp8 DoubleRow
           (w2Ts input is fp8 x8 => C carries x64); bands 0+1 and 2+3 packed
           into two fp8 AllReduces overlapped with compute; lower triangle
           mirrored locally via PE transposes; PT1 chunked per AR pair.
  S1: A1 = fp8(W1*64)*m1, PT = C@A1T (DoubleRow, psum/16 -> fp8 x256),
      G = PT^T A1T (DoubleRow, psum = 16384x), Prec = G*invsp2/16384 + st + I,
      LDLT/Lt^-1/dz/tr/logdet in bf16, z_s -> AllGather (1KB)
  S2: h2 (fp8), x_star = h2 @ w2s(x8) DoubleRow (delta = xmb - xs/8 fused),
      d_sq, Wd = dT8 @ w2Ts(x8) DoubleRow, t_part -> AllGather [t|d_sq] 17KB
  S3: G2 = A2 C A2^T (DoubleRow), LDLT2 (bf16), fwd solve,
      d_proj = (sum y^2/D2)/64, recon.

Identities: Prec = Lt D Lt^T; sum(log eig) = sum(log D);
  sum(1/eig) = ||D^-.5 Lt^-1||_F^2;  U^-1 eps = Lt^-T (eps/sqrt(D));
  t^T G2^-1 t = ||D2^-.5 Lt2^-1 t||^2.  After LDLT the strict upper of T
  mirrors the unscaled u columns (trailing blocks stay symmetric).
"""
import sys

for _p in ("/opt/trn_rl_repo", "/root/.axon_site/_ro/trn_rl_repo"):
    if _p not in sys.path:
        sys.path.append(_p)

import numpy as np
from contextlib import ExitStack

import concourse.bass as bass
import concourse.mybir as mybir
import concourse.tile as tile
from concourse.masks import make_identity

B, D, N, H = 64, 12288, 32, 2048
NCORES = 8
BL = B // NCORES          # 8 local samples
HS = H // NCORES          # 256
DS = D // NCORES          # 1536
KT_H = H // 128           # 16
KT_D = D // 128           # 96
KT_DS = DS // 128         # 12
P = 128
EC = 8                    # encoder k chunks
EK = KT_D // EC           # 12 k-tiles per encoder chunk

SCW = 4.0                 # host pre-scale on fp8 w2Ts/w2s => C carries x16
SA = 64.0                 # host pre-scale on w1T8 (fp8 A)
SPT = 1.0 / 8.0           # PT psum(1024x) -> fp8 x128
SG = 128.0 * SA           # G psum scale = 8192x
ST2 = SCW * SCW           # t carries x4 -> dproj x16

F32 = mybir.dt.float32
BF16 = mybir.dt.bfloat16
FP8 = mybir.dt.float8e4
DR = mybir.MatmulPerfMode.DoubleRow
Alu = mybir.AluOpType
Act = mybir.ActivationFunctionType
RG = [list(range(NCORES))]


def sub_ap(t, extra_off, dims):
    """Custom free-dim AP on a [P, F] tile; dims = [[step,count],...] in elems."""
    base = t[:, 0:1]
    return bass.AP(base.tensor, base.offset + extra_off, [base.ap[0]] + dims)


def pe_T(nc, out_ps, in_ap, ident):
    """PE transpose: out_ps [f, p] = in_ap [p, f].T"""
    kp = in_ap.shape[0]
    nc.tensor.transpose(out_ps, in_ap, ident[0:kp, 0:kp])


def emit_ldlt_ltinv(nc, T, OUT, OUT2, X, invD, n=32):
    """Fused stage-1 LDLT + Lt^{-1}: ltinv step j-1 interleaves with LDLT
    iter j (its inputs are final by then), filling dependency-wait gaps on
    the in-order vector queue. ltinv sign folded into the subtract."""
    def ltinv_step(k):
        rows = n - 1 - k
        cols = k + 1
        urow = sub_ap(T, k * n + k + 1, [[1, rows], [0, cols]])
        xrow = sub_ap(X, k * n, [[0, rows], [1, cols]])
        prod = sub_ap(OUT2, 0, [[cols, rows], [1, cols]])
        nc.vector.scalar_tensor_tensor(
            prod, urow, invD[:, k:k + 1], xrow, Alu.mult, Alu.mult)
        xblk = sub_ap(X, (k + 1) * n, [[n, rows], [1, cols]])
        nc.vector.tensor_tensor(xblk, xblk, prod, Alu.subtract)

    for j in range(n):
        nc.vector.reciprocal(invD[:, j:j + 1], T[:, (n + 1) * j:(n + 1) * j + 1])
        m = n - 1 - j
        if m > 0:
            u_i = sub_ap(T, (j + 1) * n + j, [[n, m], [0, m]])
            u_k = sub_ap(T, j * n + j + 1, [[0, m], [1, m]])
            outer = sub_ap(OUT, 0, [[m, m], [1, m]])
            nc.vector.scalar_tensor_tensor(
                outer, u_i, invD[:, j:j + 1], u_k, Alu.mult, Alu.mult)
            trail = sub_ap(T, (j + 1) * (n + 1), [[n, m], [1, m]])
            nc.vector.tensor_tensor(trail, trail, outer, Alu.subtract)
        if 1 <= j < n:
            ltinv_step(j - 1)


def emit_ldlt(nc, T, OUT, invD, n=32):
    """In-place unit-lower LDLT of T [BL, n*n] (row-major per sample).
    After: strict lower of T holds unscaled columns u; diag holds D; invD = 1/D.
    u_k read from row j (symmetric trailing block) for unit-stride access."""
    for j in range(n):
        nc.vector.reciprocal(invD[:, j:j + 1], T[:, (n + 1) * j:(n + 1) * j + 1])
        m = n - 1 - j
        if m == 0:
            break
        u_i = sub_ap(T, (j + 1) * n + j, [[n, m], [0, m]])
        u_k = sub_ap(T, j * n + j + 1, [[0, m], [1, m]])
        outer = sub_ap(OUT, 0, [[m, m], [1, m]])
        nc.vector.scalar_tensor_tensor(
            outer, u_i, invD[:, j:j + 1], u_k, Alu.mult, Alu.mult)
        trail = sub_ap(T, (j + 1) * (n + 1), [[n, m], [1, m]])
        nc.vector.tensor_tensor(trail, trail, outer, Alu.subtract)


def emit_ltinv(nc, T, negD, X, OUT, n=32):
    """X = Lt^{-1}; column scaling folded via negD = -invD; u row-read."""
    for k in range(n - 1):
        rows = n - 1 - k
        cols = k + 1
        urow = sub_ap(T, k * n + k + 1, [[1, rows], [0, cols]])
        xrow = sub_ap(X, k * n, [[0, rows], [1, cols]])
        prod = sub_ap(OUT, 0, [[cols, rows], [1, cols]])
        nc.vector.scalar_tensor_tensor(
            prod, urow, negD[:, k:k + 1], xrow, Alu.mult, Alu.mult)
        xblk = sub_ap(X, (k + 1) * n, [[n, rows], [1, cols]])
        nc.vector.tensor_tensor(xblk, xblk, prod, Alu.add)


def emit_fwd_solve(nc, T, negD, y, sk, n=32):
    """y <- Lt^{-1} y; per-step scalar s_k = -y_k/D_k folded so each step is
    one tiny [BL,1] mult plus one in-place STT."""
    for k in range(n - 1):
        rows = n - 1 - k
        urow = sub_ap(T, k * n + k + 1, [[1, rows]])
        nc.vector.tensor_tensor(sk, negD[:, k:k + 1], y[:, k:k + 1], Alu.mult)
        nc.vector.scalar_tensor_tensor(
            y[:, k + 1:n], urow, sk, y[:, k + 1:n], Alu.mult, Alu.add)


def legalize_waits(nc, maxw=1):
    """Split multi-wait sync_info into standalone EventSemaphore instructions."""
    for f in nc.m.functions:
        for bb in f.blocks:
            insts = list(bb.instructions)
            out = []
            changed = False
            for inst in insts:
                si = inst.sync_info
                if si is not None and si.on_wait and len(si.on_wait) > maxw:
                    waits = list(si.on_wait)
                    imm = [w for w in waits if w.uses_immediate]
                    reg = [w for w in waits if not w.uses_immediate]
                    keep = (reg + imm)[:maxw] if len(reg) <= maxw else reg
                    extra = [w for w in waits if w not in keep]
                    if len(keep) > maxw:
                        raise RuntimeError(f"{inst.name}: {len(keep)} register waits")
                    for w in extra:
                        ev = mybir.InstEventSemaphore(
                            name=nc.get_next_instruction_name(), ins=[], outs=[])
                        ev.engine = inst.engine
                        ev.sync_info = mybir.SyncInfo(on_wait=[w], on_update=[])
                        out.append(ev)
                    inst.sync_info = mybir.SyncInfo(
                        on_wait=keep, on_update=list(si.on_update or []))
                    changed = True
                out.append(inst)
            if changed:
                bb.instructions = out
    return nc


def build_nc():
    nc = bass.Bass()

    # ---- I/O ----
    xT = nc.dram_tensor("xT", [D, B], BF16, kind="ExternalInput")
    xmb = nc.dram_tensor("xmb", [B, DS], F32, kind="ExternalInput")
    w1es = nc.dram_tensor("w1es", [D, HS], BF16, kind="ExternalInput")
    b1es = nc.dram_tensor("b1es", [1, HS], BF16, kind="ExternalInput")
    w2es = nc.dram_tensor("w2es", [HS, N], BF16, kind="ExternalInput")
    b2e = nc.dram_tensor("b2e", [1, N], F32, kind="ExternalInput")
    w2Ts = nc.dram_tensor("w2Ts", [DS, H], FP8, kind="ExternalInput")   # x8
    w2s = nc.dram_tensor("w2s", [H, DS], FP8, kind="ExternalInput")     # x8
    w1 = nc.dram_tensor("w1", [N, H], BF16, kind="ExternalInput")
    w1Td = nc.dram_tensor("w1Td", [H, N], BF16, kind="ExternalInput")
    w1T8d = nc.dram_tensor("w1T8d", [H, N], FP8, kind="ExternalInput")  # x64
    b1d = nc.dram_tensor("b1d", [1, H], BF16, kind="ExternalInput")
    sigw = nc.dram_tensor("sigw", [1, 130], F32, kind="ExternalInput")
    sel8 = nc.dram_tensor("sel8", [B, BL], F32, kind="ExternalInput")
    epsin = nc.dram_tensor("epsin", [BL, N], F32, kind="ExternalInput")
    out = nc.dram_tensor("out", [BL, 1], F32, kind="ExternalOutput")
    dbg = nc.dram_tensor("dbg", [BL, 48], F32, kind="ExternalOutput")

    # ---- internal DRAM ----
    dum_b = nc.dram_tensor("dum_b", [1, 16], F32)
    dum_sh = nc.dram_tensor("dum_sh", [NCORES, 16], F32, addr_space="Shared")
    zst_b = nc.dram_tensor("zst_b", [B, N], F32)
    zst8_sh = nc.dram_tensor("zst8_sh", [NCORES * B, N], F32, addr_space="Shared")
    # bands 0+1 packed: [512 rows, 2048 (b0) + 1536 (b1)]; bands 2+3:
    # [512 rows, 1024 (b2) + 512 (b3)]
    cpart01 = nc.dram_tensor("cpart01", [512, 3584], FP8)
    c_sh01 = nc.dram_tensor("c_sh01", [512, 3584], FP8, addr_space="Shared")
    cpart23 = nc.dram_tensor("cpart23", [512, 1536], FP8)
    c_sh23 = nc.dram_tensor("c_sh23", [512, 1536], FP8, addr_space="Shared")
    zs_b = nc.dram_tensor("zs_b", [BL, N], F32)
    zs_sh = nc.dram_tensor("zs_sh", [B, N], F32, addr_space="Shared")
    tar_b = nc.dram_tensor("tar_b", [B, N + 1], F32)
    tar_sh = nc.dram_tensor("tar_sh", [NCORES * B, N + 1], F32, addr_space="Shared")

    with tile.TileContext(nc) as tc, ExitStack() as ctx:
        consts = ctx.enter_context(tc.tile_pool(name="consts", bufs=1))
        work = ctx.enter_context(tc.tile_pool(name="work", bufs=2))
        stream = ctx.enter_context(tc.tile_pool(name="stream", bufs=4))
        res = ctx.enter_context(tc.tile_pool(name="res", bufs=1))
        csb = ctx.enter_context(tc.tile_pool(name="csb", bufs=1))
        pC = ctx.enter_context(tc.tile_pool(name="pC", bufs=1))
        psum = ctx.enter_context(tc.tile_pool(name="psum", bufs=2, space="PSUM"))
        psum_acc = ctx.enter_context(tc.tile_pool(name="psacc", bufs=1, space="PSUM"))
        lin = ctx.enter_context(tc.tile_pool(name="lin", bufs=1))

        # ---- w2Ts loads first on sync queue (C compute is the PE pole) ----
        w2Ts_sb = pC.tile([P, KT_DS, H], FP8, tag="w2Ts")
        w2Ts_r = w2Ts[:].rearrange("(k p) h -> p k h", p=P)
        for kt in range(KT_DS):
            (nc.sync if kt % 2 else nc.scalar).dma_start(
                w2Ts_sb[:, kt, :], w2Ts_r[:, kt, :])

        # ---- dummy first collective: absorbs the cross-core start barrier ----
        dum_sb = consts.tile([1, 16], F32)
        nc.vector.memset(dum_sb, 0.0)
        nc.scalar.dma_start(dum_b[:], dum_sb)
        nc.gpsimd.collective_compute("AllGather", Alu.bypass, replica_groups=RG,
                                     ins=[dum_b[:]], outs=[dum_sh[:]])

        # ---- constants / small loads ----
        ident = consts.tile([P, P], F32)
        make_identity(nc, ident)
        identb = consts.tile([P, P], BF16)
        make_identity(nc, identb)
        ident8 = consts.tile([P, P], FP8)
        make_identity(nc, ident8)
        ones1 = consts.tile([1, B], F32)
        nc.vector.memset(ones1, 1.0)
        onesb = consts.tile([1, B], BF16)
        nc.vector.memset(onesb, 1.0)
        sigw_sb = consts.tile([1, 130], F32)
        nc.sync.dma_start(sigw_sb, sigw[:])
        sel8_sb = consts.tile([B, BL], F32)
        nc.sync.dma_start(sel8_sb, sel8[:])
        eps_sb = consts.tile([BL, N], F32)
        nc.sync.dma_start(eps_sb, epsin[:])
        b1es_sb = consts.tile([1, HS], BF16)
        nc.sync.dma_start(b1es_sb, b1es[:])
        b2e_sb = consts.tile([1, N], F32)
        nc.sync.dma_start(b2e_sb, b2e[:])
        b1d_sb = consts.tile([1, H], BF16)
        nc.sync.dma_start(b1d_sb, b1d[:])
        w1_sb = consts.tile([N, H], BF16)
        nc.sync.dma_start(w1_sb, w1[:])
        w1T_sb = consts.tile([P, KT_H, N], BF16)
        nc.sync.dma_start(w1T_sb, w1Td[:].rearrange("(k p) n -> p k n", p=P))
        w1T8_sb = consts.tile([P, KT_H, N], FP8)
        nc.sync.dma_start(w1T8_sb, w1T8d[:].rearrange("(k p) n -> p k n", p=P))
        w2es_sb = consts.tile([P, 2, N], BF16)
        nc.sync.dma_start(w2es_sb, w2es[:].rearrange("(k p) n -> p k n", p=P))

        C_sb = csb.tile([P, KT_H, H], FP8, tag="C")

        # ============ phase C: upper row bands, fp8 DoubleRow ========
        def band_dst(g, rl, nb):
            if g == 0:
                return cpart01, rl * P, nb * 512
            if g == 1:
                return cpart01, rl * P, 2048 + (nb - 1) * 512
            if g == 2:
                return cpart23, rl * P, (nb - 2) * 512
            return cpart23, rl * P, 1024 + (nb - 3) * 512

        wq = [nc.scalar, nc.sync]

        def emit_band_tiles(g):
            for rl in range(4):
                r = 4 * g + rl
                for nb in range(g, 4):
                    cps = psum.tile([P, 512], F32, tag="big_ps")
                    for q in range(KT_DS // 2):
                        nc.tensor.matmul(
                            cps,
                            w2Ts_sb[:, 2 * q:2 * q + 2, r * P:(r + 1) * P],
                            w2Ts_sb[:, 2 * q:2 * q + 2, nb * 512:(nb + 1) * 512],
                            start=(q == 0), stop=(q == KT_DS // 2 - 1),
                            perf_mode=DR)
                    cs = stream.tile([P, 512], FP8, tag="c_out")
                    nc.scalar.copy(cs, cps)
                    ten, ro, co = band_dst(g, rl, nb)
                    wq[(rl + nb) % 2].dma_start(
                        ten[ro:ro + P, co:co + 512], cs)

        emit_band_tiles(0)
        emit_band_tiles(1)
        nc.gpsimd.collective_compute("AllReduce", Alu.add, replica_groups=RG,
                                     ins=[cpart01[:]], outs=[c_sh01[:]])
        for rl in range(4):
            nc.scalar.dma_start(C_sb[:, rl, :],
                                c_sh01[rl * P:(rl + 1) * P, 0:2048])
            nc.sync.dma_start(C_sb[:, 4 + rl, 512:],
                              c_sh01[rl * P:(rl + 1) * P, 2048:3584])
        emit_band_tiles(2)
        emit_band_tiles(3)
        nc.gpsimd.collective_compute("AllReduce", Alu.add, replica_groups=RG,
                                     ins=[cpart23[:]], outs=[c_sh23[:]])
        for rl in range(4):
            nc.scalar.dma_start(C_sb[:, 8 + rl, 1024:],
                                c_sh23[rl * P:(rl + 1) * P, 0:1024])
            nc.sync.dma_start(C_sb[:, 12 + rl, 1536:],
                              c_sh23[rl * P:(rl + 1) * P, 1024:1536])

        # ================= phase E: encoder =================
        pE_cm = tc.tile_pool(name="pE", bufs=3)
        pE = pE_cm.__enter__()
        a1_ps = psum_acc.tile([B, HS], F32, tag="acc")
        for c in range(EC):
            xt_c = pE.tile([P, EK, B], BF16, tag="xt")
            nc.scalar.dma_start(
                xt_c, xT[c * EK * P:(c + 1) * EK * P, :].rearrange(
                    "(k p) b -> p k b", p=P))
            w1c = pE.tile([P, EK, HS], BF16, tag="w1c")
            (nc.sync if c % 2 else nc.scalar).dma_start(
                w1c, w1es[c * EK * P:(c + 1) * EK * P, :].rearrange(
                    "(k p) h -> p k h", p=P))
            for kt in range(EK):
                nc.tensor.matmul(a1_ps, xt_c[:, kt, :], w1c[:, kt, :],
                                 start=(c == 0 and kt == 0), stop=False)
        nc.tensor.matmul(a1_ps, onesb[:, 0:B], b1es_sb, start=False, stop=True)
        h1_sb = work.tile([B, HS], BF16, tag="h1")
        nc.vector.tensor_scalar(h1_sb, a1_ps, 0.0, None, Alu.max)
        h1T_sb = work.tile([P, 2, B], BF16, tag="h1T")
        for i in range(2):
            tp = psum.tile([P, B], BF16, tag="small_ps")
            pe_T(nc, tp, h1_sb[:, i * P:(i + 1) * P], identb)
            nc.vector.tensor_copy(h1T_sb[:, i, :], tp)
        pE_cm.__exit__(None, None, None)
        zp_ps = psum.tile([B, N], F32, tag="small_ps")
        for i in range(2):
            nc.tensor.matmul(zp_ps, h1T_sb[:, i, :], w2es_sb[:, i, :],
                             start=(i == 0), stop=(i == 1))
        zp_sb = work.tile([B, N], F32, tag="zstar_part")
        nc.vector.tensor_copy(zp_sb, zp_ps)
        nc.gpsimd.dma_start(zst_b[:], zp_sb)
        nc.gpsimd.collective_compute("AllGather", Alu.bypass, replica_groups=RG,
                                     ins=[zst_b[:]], outs=[zst8_sh[:]])

        # ---- mirrors (all; C fully AllReduced by the time PE gets here) ----
        mirrorsA = [(R, C) for R in range(4, KT_H) for C in range(min(4 * (R // 4), 8))]
        mirrorsB = [(R, C) for R in range(12, KT_H) for C in range(8, 4 * (R // 4))]

        def emit_mirrors(targets):
            # target (R, C128) <- transpose of C_sb[:, C128, R-block]
            # (fp8 transpose writes with element step 2 -> strided psum view)
            for R, C128 in targets:
                mt_ps = psum.tile([P, 2 * P], FP8, tag="small_ps")
                mt_v = sub_ap(mt_ps, 0, [[2, P]])
                pe_T(nc, mt_v, C_sb[:, C128, R * P:(R + 1) * P], ident8)
                nc.scalar.copy(C_sb[:, R, C128 * P:(C128 + 1) * P], mt_v)

        emit_mirrors(mirrorsA)
        emit_mirrors(mirrorsB)

        # ========== z* post: local slice, sig1, masks, A1T ======
        z8 = work.tile([B, NCORES, N], F32, tag="z8")
        nc.gpsimd.dma_start(z8, zst8_sh[:].rearrange("(c b) n -> b c n", b=B))
        zf_sb = work.tile([B, N], F32, tag="z_full")
        nc.vector.tensor_reduce(
            zf_sb, sub_ap(z8, 0, [[1, N], [N, NCORES]]),
            mybir.AxisListType.X, Alu.add)
        zloc_ps = psum.tile([BL, N], F32, tag="small_ps")
        nc.tensor.matmul(zloc_ps, sel8_sb, zf_sb, start=True, stop=False)
        nc.tensor.matmul(zloc_ps, ones1[:, 0:BL], b2e_sb, start=False, stop=True)
        zloc_sb = lin.tile([BL, N], F32, tag="z_loc")   # z* local + b2
        nc.vector.tensor_copy(zloc_sb, zloc_ps)

        sigw_rep = consts.tile([BL, 130], F32)
        sigw_ps = psum.tile([BL, 130], F32, tag="small_ps")
        nc.tensor.matmul(sigw_ps, ones1[:, 0:BL], sigw_sb, start=True, stop=True)
        nc.vector.tensor_copy(sigw_rep, sigw_ps)

        def emit_sig(z_loc, name):
            lg = lin.tile([BL, 2, 32], F32, tag="sig_lg")
            nc.vector.tensor_tensor(
                lg, z_loc.unsqueeze(1).broadcast_to([BL, 2, 32]),
                sigw_rep[:, 0:64].rearrange("p (c n) -> p c n", c=2), Alu.mult)
            red = lin.tile([BL, 2], F32, tag=f"sig_red_{name}")
            nc.vector.tensor_reduce(red, lg, mybir.AxisListType.X, Alu.add)
            nc.vector.tensor_tensor(red, red, sigw_rep[:, 64:66], Alu.add)
            s = lin.tile([BL, 2], F32, tag=f"sig_s_{name}")
            nc.scalar.activation(s, red, Act.Exp)
            return s

        s1 = emit_sig(zloc_sb, "s1")
        invsp2 = lin.tile([BL, 1], F32, tag="invsp2")
        sp2t = lin.tile([BL, 1], F32, tag="sp2t")
        nc.vector.tensor_tensor(sp2t, s1[:, 0:1], s1[:, 0:1], Alu.mult)
        nc.vector.reciprocal(invsp2, sp2t)
        nc.vector.tensor_scalar(invsp2, invsp2, 1.0 / SG, None, Alu.mult)

        zlT_ps = psum.tile([N, BL], F32, tag="small_ps")
        pe_T(nc, zlT_ps, zloc_sb, ident)
        zlTb = work.tile([N, BL], BF16, tag="zlT")
        nc.vector.tensor_copy(zlTb, zlT_ps)

        # a1T (local) -> mask m1T (fp8) -> A1T = w1T8 * m1T (fp8, x64)
        a1T_ps = psum_acc.tile([P, KT_H, BL], F32, tag="acc")
        for mt in range(KT_H):
            nc.tensor.matmul(a1T_ps[:, mt, :], w1_sb[:, mt * P:(mt + 1) * P],
                             zlTb, start=True, stop=False)
            nc.tensor.matmul(a1T_ps[:, mt, :], b1d_sb[:, mt * P:(mt + 1) * P],
                             onesb[:, 0:BL], start=False, stop=True)
        m1T_sb = work.tile([P, KT_H, BL], FP8, tag="m1T")
        nc.vector.tensor_scalar(m1T_sb, a1T_ps, 0.0, None, Alu.is_gt)
        AT_sb = res.tile([P, KT_H, BL, N], FP8, tag="AT")
        nc.vector.tensor_tensor(
            AT_sb,
            w1T8_sb.unsqueeze(2).broadcast_to([P, KT_H, BL, N]),
            m1T_sb.unsqueeze(3).broadcast_to([P, KT_H, BL, N]), Alu.mult)

        # sig_term (constant across batch); staged via an f32 scratch row
        stf = lin.tile([1, N * N], F32, tag="stf")
        st_ps = psum.tile([N, N], F32, tag="small_ps")
        nc.tensor.matmul(st_ps, sigw_sb[:, 66:98], sigw_sb[:, 66:98],
                         start=True, stop=False)
        nc.tensor.matmul(st_ps, sigw_sb[:, 98:130], sigw_sb[:, 98:130],
                         start=False, stop=True)
        st_sb = work.tile([N, N], F32, tag="st_sb")
        nc.vector.tensor_copy(st_sb, st_ps)
        nc.sync.dma_start(stf, st_sb)
        st_rep = lin.tile([BL, N * N], BF16, tag="st_rep")
        for hh in range(2):
            st_ps2 = psum.tile([BL, 512], F32, tag="big_ps")
            nc.tensor.matmul(st_ps2, ones1[:, 0:BL],
                             stf[:, hh * 512:(hh + 1) * 512], start=True, stop=True)
            nc.vector.tensor_copy(st_rep[:, hh * 512:(hh + 1) * 512], st_ps2)

        def emit_PT(AT, PT_sb, mts):
            for mt in mts:
                pps = psum.tile([P, BL * N], F32, tag="mid_ps")
                for q in range(KT_H // 2):
                    nc.tensor.matmul(
                        pps, C_sb[:, 2 * q:2 * q + 2, mt * P:(mt + 1) * P],
                        AT[:, 2 * q:2 * q + 2, :, :],
                        start=(q == 0), stop=(q == KT_H // 2 - 1), perf_mode=DR)
                nc.scalar.mul(PT_sb[:, mt, :], pps, SPT)

        def emit_G(AT, PT_sb, Tdst):
            g_ps = psum_acc.tile([N, BL * N], F32, tag="acc")
            g_sb = work.tile([N, BL * N], BF16, tag="g_sb")
            for s in range(BL):
                for q in range(KT_H // 2):
                    nc.tensor.matmul(
                        g_ps[:, s * N:(s + 1) * N],
                        PT_sb[:, 2 * q:2 * q + 2, s * N:(s + 1) * N],
                        AT[:, 2 * q:2 * q + 2, s, :],
                        start=(q == 0), stop=(q == KT_H // 2 - 1), perf_mode=DR)
                nc.vector.tensor_copy(g_sb[:, s * N:(s + 1) * N],
                                      g_ps[:, s * N:(s + 1) * N])
                nc.sync.dma_start(Tdst[s:s + 1, :], g_sb[:, s * N:(s + 1) * N])

        PT1_sb = res.tile([P, KT_H, BL * N], FP8, tag="PT")
        emit_PT(AT_sb, PT1_sb, range(KT_H))
        Tm = lin.tile([BL, N * N], BF16, tag="Tmat")
        emit_G(AT_sb, PT1_sb, Tm)

        # w2s resident loads (needed at x_star time; queues idle during LDLT)
        w2s_sb = pC.tile([P, KT_H, DS], FP8, tag="w2s")
        w2s_r0 = w2s[:].rearrange("(k p) ds -> p k ds", p=P)
        for kt in range(KT_H):
            (nc.sync if kt % 2 else nc.scalar).dma_start(
                w2s_sb[:, kt, :], w2s_r0[:, kt, :])

        # ---- Prec = G*invsp2/SG + sig_term + I  (bf16) ----
        nc.vector.tensor_scalar(Tm, Tm, invsp2, None, Alu.mult)
        nc.vector.tensor_tensor(Tm, Tm, st_rep, Alu.add)
        diag1 = sub_ap(Tm, 0, [[N + 1, N]])
        nc.vector.tensor_scalar(diag1, diag1, 1.0, None, Alu.add)

        # ---- LDLT, Lt^-1, dz, tr, ld  (bf16 chain) ----
        invD = lin.tile([BL, N], F32, tag="invD")
        SCR = lin.tile([BL, N * N], BF16, tag="scr")
        SCR2 = lin.tile([BL, N * N], BF16, tag="scr2")
        X1 = lin.tile([BL, N * N], BF16, tag="X1")
        nc.vector.memset(X1, 0.0)
        nc.vector.memset(sub_ap(X1, 0, [[N + 1, N]]), 1.0)
        emit_ldlt_ltinv(nc, Tm, SCR, SCR2, X1, invD)

        srD = lin.tile([BL, N], F32, tag="srD")
        nc.scalar.activation(srD, invD, Act.Sqrt)        # 1/sqrt(D)
        epss = lin.tile([BL, N], BF16, tag="epss")
        nc.vector.tensor_tensor(epss, eps_sb, srD, Alu.mult)
        nc.vector.tensor_tensor(
            SCR.rearrange("p (a b) -> p a b", b=N),
            X1.rearrange("p (a b) -> p a b", b=N),
            epss.unsqueeze(2).broadcast_to([BL, N, N]), Alu.mult)
        dz = lin.tile([BL, N], F32, tag="dz")
        nc.vector.tensor_reduce(
            dz, sub_ap(SCR, 0, [[1, N], [N, N]]), mybir.AxisListType.X, Alu.add)
        zs_loc = lin.tile([BL, N], F32, tag="zs_loc")
        nc.vector.tensor_tensor(zs_loc, zloc_sb, dz, Alu.add)
        nc.gpsimd.dma_start(zs_b[:], zs_loc)
        nc.gpsimd.collective_compute("AllGather", Alu.bypass, replica_groups=RG,
                                     ins=[zs_b[:]], outs=[zs_sh[:]])
        # tr = sum X1^2 * invD_row
        nc.vector.tensor_tensor(SCR, X1, X1, Alu.mult)
        nc.vector.tensor_tensor(
            SCR.rearrange("p (a b) -> p a b", b=N),
            SCR.rearrange("p (a b) -> p a b", b=N),
            invD.unsqueeze(2).broadcast_to([BL, N, N]), Alu.mult)
        trv = lin.tile([BL, 1], F32, tag="trv")
        nc.vector.tensor_reduce(trv, SCR, mybir.AxisListType.X, Alu.add)
        logs = lin.tile([BL, N], F32, tag="logs")
        ldv = lin.tile([BL, 1], F32, tag="ldv")
        nc.scalar.activation(logs, invD, Act.Ln)
        nc.vector.tensor_reduce(ldv, logs, mybir.AxisListType.X, Alu.add)  # -sum log D
        zsq = lin.tile([BL, N], F32, tag="zsq")
        latv = lin.tile([BL, 1], F32, tag="latv")
        nc.vector.tensor_tensor(zsq, zloc_sb, zloc_sb, Alu.mult)
        nc.vector.tensor_reduce(latv, zsq, mybir.AxisListType.X, Alu.add)
        nc.vector.tensor_tensor(latv, latv, trv, Alu.add)
        nc.vector.tensor_scalar(latv, latv, 0.5, None, Alu.mult)

        # ---- stage 2 masks (local z_s only; independent of the AllGather) ----
        zslT_ps = psum.tile([N, BL], F32, tag="small_ps")
        pe_T(nc, zslT_ps, zs_loc, ident)
        zslTb = work.tile([N, BL], BF16, tag="zlT")
        nc.vector.tensor_copy(zslTb, zslT_ps)
        a2lT_ps = psum.tile([P, KT_H, BL], F32, tag="small_ps")
        for mt in range(KT_H):
            nc.tensor.matmul(a2lT_ps[:, mt, :], w1_sb[:, mt * P:(mt + 1) * P],
                             zslTb, start=True, stop=False)
            nc.tensor.matmul(a2lT_ps[:, mt, :], b1d_sb[:, mt * P:(mt + 1) * P],
                             onesb[:, 0:BL], start=False, stop=True)
        m2T_sb = work.tile([P, KT_H, BL], FP8, tag="m1T")
        nc.vector.tensor_scalar(m2T_sb, a2lT_ps, 0.0, None, Alu.is_gt)
        AT2_sb = res.tile([P, KT_H, BL, N], FP8, tag="AT")   # reuse slot
        nc.vector.tensor_tensor(
            AT2_sb,
            w1T8_sb.unsqueeze(2).broadcast_to([P, KT_H, BL, N]),
            m2T_sb.unsqueeze(3).broadcast_to([P, KT_H, BL, N]), Alu.mult)

        # ---- z_s-only recon terms (run during stage-2 PE window) ----
        s2 = emit_sig(zs_loc, "s2")
        sq2 = lin.tile([BL, 2], F32, tag="sq2")
        nc.vector.tensor_tensor(sq2, s2, s2, Alu.mult)
        nc.vector.tensor_scalar(sq2, sq2, 2.0, None, Alu.mult)
        inv2 = lin.tile([BL, 2], F32, tag="inv2")
        nc.vector.reciprocal(inv2, sq2)     # [1/(2sp2^2), 1/(2sv2^2)]
        logs2 = lin.tile([BL, 2], F32, tag="logs2")
        logw = lin.tile([BL, 2], F32, tag="logw")
        nc.vector.memset(logw[:, 0:1], float(N))
        nc.vector.memset(logw[:, 1:2], float(D - N))
        nc.scalar.activation(logs2, s2, Act.Ln)
        logterm = lin.tile([BL, 1], F32, tag="logterm")
        junk2 = lin.tile([BL, 2], F32, tag="junk2")
        nc.vector.tensor_tensor(junk2, logs2, logw, Alu.mult)
        nc.vector.tensor_reduce(logterm, junk2, mybir.AxisListType.X, Alu.add)
        isub = lin.tile([BL, 1], F32, tag="isub")
        nc.vector.tensor_tensor(isub, inv2[:, 0:1], inv2[:, 1:2], Alu.subtract)
        # rest = logterm + latent + logdet/2  (everything not needing t)
        nc.vector.tensor_scalar(ldv, ldv, -0.5, None, Alu.mult)
        rest = lin.tile([BL, 1], F32, tag="rest")
        nc.vector.tensor_tensor(rest, logterm, latv, Alu.add)
        nc.vector.tensor_tensor(rest, rest, ldv, Alu.add)

        # ---- G2, LDLT2 (vector grinds while PE does the x_star path) ----
        PT2_sb = res.tile([P, KT_H, BL * N], FP8, tag="PT")   # reuse slot
        emit_PT(AT2_sb, PT2_sb, range(KT_H))
        Tm2 = lin.tile([BL, N * N], BF16, tag="Tmat")   # reuse slot
        emit_G(AT2_sb, PT2_sb, Tm2)
        nc.vector.tensor_scalar(Tm2, Tm2, 1.0 / SG, None, Alu.mult)
        invD2 = lin.tile([BL, N], F32, tag="invD2")
        emit_ldlt(nc, Tm2, SCR, invD2)
        negD2 = lin.tile([BL, N], F32, tag="negD2")
        nc.vector.tensor_scalar(negD2, invD2, -1.0, None, Alu.mult)

        # ---- full-batch h2/masks, x_star slice, delta, d_sq, Wd, t_part ----
        post = ctx.enter_context(tc.tile_pool(name="post", bufs=1))
        zsf_sb = work.tile([B, N], F32, tag="z_full")
        nc.gpsimd.dma_start(zsf_sb, zs_sh[:])
        zsT_ps = psum.tile([N, B], F32, tag="small_ps")
        pe_T(nc, zsT_ps, zsf_sb, ident)
        zsTb = work.tile([N, B], BF16, tag="zT")
        nc.vector.tensor_copy(zsTb, zsT_ps)
        a2T_ps = psum_acc.tile([P, KT_H, B], F32, tag="acc")
        for mt in range(KT_H):
            nc.tensor.matmul(a2T_ps[:, mt, :], w1_sb[:, mt * P:(mt + 1) * P],
                             zsTb, start=True, stop=False)
            nc.tensor.matmul(a2T_ps[:, mt, :], b1d_sb[:, mt * P:(mt + 1) * P],
                             onesb[:, 0:B], start=False, stop=True)
        h2T_sb = post.tile([P, KT_H, B], FP8, tag="h2T")
        nc.scalar.activation(h2T_sb, a2T_ps, Act.Relu)
        m2f_sb = post.tile([P, KT_H, B], BF16, tag="m2f")
        nc.vector.tensor_scalar(m2f_sb, a2T_ps, 0.0, None, Alu.is_gt)

        d_sb = post.tile([B, DS], BF16, tag="d_sb")
        dsqp = work.tile([B, 3], F32, tag="dsqp")
        xmb_ts = []
        for nb in range(3):
            xmb_t = stream.tile([B, 512], F32, tag=f"xmb_t{nb}")
            nc.sync.dma_start(xmb_t, xmb[:, nb * 512:(nb + 1) * 512])
            xmb_ts.append(xmb_t)
        for nb in range(3):
            xmb_t = xmb_ts[nb]
            xs_ps = psum.tile([B, 512], F32, tag="big_ps")
            for q in range(KT_H // 2):
                nc.tensor.matmul(xs_ps, h2T_sb[:, 2 * q:2 * q + 2, :],
                                 w2s_sb[:, 2 * q:2 * q + 2, nb * 512:(nb + 1) * 512],
                                 start=(q == 0), stop=(q == KT_H // 2 - 1),
                                 perf_mode=DR)
            # delta = xmb - xs/8  (w2s carries x8)
            nc.vector.scalar_tensor_tensor(
                d_sb[:, nb * 512:(nb + 1) * 512], xs_ps, -1.0 / SCW, xmb_t,
                Alu.mult, Alu.add)
            dsqf = work.tile([B, 512], F32, tag="dsqf")
            nc.scalar.activation(dsqf, d_sb[:, nb * 512:(nb + 1) * 512], Act.Square)
            nc.vector.tensor_reduce(dsqp[:, nb:nb + 1], dsqf,
                                    mybir.AxisListType.X, Alu.add)
        dsq_sb = work.tile([B, 1], F32, tag="dsq")
        nc.vector.tensor_reduce(dsq_sb, dsqp, mybir.AxisListType.X, Alu.add)
        dT_sb = post.tile([P, KT_DS, B], FP8, tag="dT")
        for kt in range(KT_DS):
            tp = psum.tile([P, B], BF16, tag="small_ps")
            pe_T(nc, tp, d_sb[:, kt * P:(kt + 1) * P], identb)
            nc.vector.tensor_copy(dT_sb[:, kt, :], tp)
        wd_sb = post.tile([B, H], BF16, tag="wd")
        for mb in range(4):
            wd_ps = psum.tile([B, 512], F32, tag="big_ps")
            for q in range(KT_DS // 2):
                nc.tensor.matmul(wd_ps, dT_sb[:, 2 * q:2 * q + 2, :],
                                 w2Ts_sb[:, 2 * q:2 * q + 2, mb * 512:(mb + 1) * 512],
                                 start=(q == 0), stop=(q == KT_DS // 2 - 1),
                                 perf_mode=DR)
            nc.vector.tensor_copy(wd_sb[:, mb * 512:(mb + 1) * 512], wd_ps)
        wdT_sb = post.tile([P, KT_H, B], BF16, tag="wdT")
        for kt in range(KT_H):
            tp2 = psum.tile([P, B], BF16, tag="small_ps")
            pe_T(nc, tp2, wd_sb[:, kt * P:(kt + 1) * P], identb)
            nc.vector.tensor_copy(wdT_sb[:, kt, :], tp2)
        mwdT_sb = post.tile([P, KT_H, B], BF16, tag="mwdT")
        nc.vector.tensor_tensor(mwdT_sb, wdT_sb, m2f_sb, Alu.mult)
        t_ps = psum.tile([N, B], F32, tag="small_ps")
        for kt in range(KT_H):
            nc.tensor.matmul(t_ps, w1T_sb[:, kt, :], mwdT_sb[:, kt, :],
                             start=(kt == 0), stop=(kt == KT_H - 1))
        tT_sb = work.tile([N, B], F32, tag="tT")
        nc.vector.tensor_copy(tT_sb, t_ps)
        tl_ps = psum.tile([B, N], F32, tag="small_ps")
        pe_T(nc, tl_ps, tT_sb, ident)
        tar_sb = work.tile([B, N + 1], F32, tag="tar")
        nc.vector.tensor_copy(tar_sb[:, 0:N], tl_ps)
        nc.vector.tensor_copy(tar_sb[:, N:N + 1], dsq_sb)
        nc.gpsimd.dma_start(tar_b[:], tar_sb)
        nc.gpsimd.collective_compute("AllGather", Alu.bypass, replica_groups=RG,
                                     ins=[tar_b[:]], outs=[tar_sh[:]])

        # ---- reduce t partials, slice local rows ----
        t8 = work.tile([B, NCORES, N + 1], F32, tag="t8")
        nc.gpsimd.dma_start(t8, tar_sh[:].rearrange("(c b) m -> b c m", b=B))
        tfull = work.tile([B, N + 1], F32, tag="tfull")
        nc.vector.tensor_reduce(
            tfull, sub_ap(t8, 0, [[1, N + 1], [N + 1, NCORES]]),
            mybir.AxisListType.X, Alu.add)
        y_ps = psum.tile([BL, N + 1], F32, tag="small_ps")
        nc.tensor.matmul(y_ps, sel8_sb, tfull, start=True, stop=True)
        y = lin.tile([BL, N], BF16, tag="y")
        nc.vector.tensor_copy(y, y_ps[:, 0:N])
        dsql = lin.tile([BL, 1], F32, tag="dsql")
        nc.vector.tensor_copy(dsql, y_ps[:, N:N + 1])

        # ---- solve G2 y = t, d_proj_sq (t carries x8 => dproj x64) ----
        sk = lin.tile([BL, 1], F32, tag="sk")
        emit_fwd_solve(nc, Tm2, negD2, y, sk)
        ysq = lin.tile([BL, N], F32, tag="ysq")
        yw = lin.tile([BL, N], F32, tag="yw")
        dproj = lin.tile([BL, 1], F32, tag="dproj")
        nc.vector.tensor_tensor(ysq, y, y, Alu.mult)
        nc.vector.tensor_tensor(yw, ysq, invD2, Alu.mult)
        nc.vector.tensor_reduce(dproj, yw, mybir.AxisListType.X, Alu.add)
        nc.vector.tensor_scalar(dproj, dproj, 1.0 / ST2, None, Alu.mult)

        # ---- recon tail (rest/logterm/isub precomputed during stage 2) ----
        recon = lin.tile([BL, 1], F32, tag="recon")
        nc.vector.tensor_tensor(recon, dproj, isub, Alu.mult)
        p2t = lin.tile([BL, 1], F32, tag="p2t")
        nc.vector.tensor_tensor(p2t, dsql, inv2[:, 1:2], Alu.mult)
        nc.vector.tensor_tensor(recon, recon, p2t, Alu.add)
        ov = lin.tile([BL, 1], F32, tag="ov")
        nc.vector.tensor_tensor(ov, recon, rest, Alu.add)
        nc.vector.tensor_scalar(ov, ov, 1.0 / D, None, Alu.mult)
        nc.sync.dma_start(out[:], ov)

    legalize_waits(nc)
    return nc


def shard_inputs(inputs):
    """Host-side prep: returns in_maps list for the 8 cores."""
    import ml_dtypes
    bf16 = ml_dtypes.bfloat16
    f8 = ml_dtypes.float8_e4m3fn

    def to_bf(a):
        return np.ascontiguousarray(np.asarray(a, np.float32).astype(bf16))

    x = np.ascontiguousarray(np.asarray(inputs["x"], np.float32))
    eps = np.ascontiguousarray(np.asarray(inputs["eps"], np.float32))
    eW1 = np.asarray(inputs["enc_W1"], np.float32)
    eb1 = np.asarray(inputs["enc_b1"], np.float32)
    eW2 = np.asarray(inputs["enc_W2"], np.float32)
    eb2 = np.asarray(inputs["enc_b2"], np.float32)
    dW1 = np.asarray(inputs["dec_W1"], np.float32)
    db1 = np.asarray(inputs["dec_b1"], np.float32)
    dW2 = np.asarray(inputs["dec_W2"], np.float32)
    db2 = np.asarray(inputs["dec_b2"], np.float32)
    sW = np.asarray(inputs["sig_W"], np.float32)
    sb = np.asarray(inputs["sig_b"], np.float32)

    xT_bf = to_bf(x.T)
    dW2T = dW2.T
    w1_bf = to_bf(dW1)
    w1Td_bf = to_bf(dW1.T)
    w1T8 = np.ascontiguousarray((dW1.T * SA).astype(f8))
    b1d_bf = to_bf(db1[None, :])
    sigv = np.zeros((1, 130), np.float32)
    sigv[0, 0:32] = sW[:, 0]
    sigv[0, 32:64] = sW[:, 1]
    sigv[0, 64:66] = sb
    sigv[0, 66:98] = sW[:, 0] * np.sqrt(N / 2.0)
    sigv[0, 98:130] = sW[:, 1] * np.sqrt((D - N) / 2.0)

    maps = []
    for k in range(NCORES):
        sel = np.zeros((B, BL), np.float32)
        for i in range(BL):
            sel[k * BL + i, i] = 1.0
        maps.append({
            "xT": xT_bf,
            "xmb": np.ascontiguousarray(
                x[:, k * DS:(k + 1) * DS] - db2[None, k * DS:(k + 1) * DS]),
            "w1es": to_bf(eW1[:, k * HS:(k + 1) * HS]),
            "b1es": to_bf(eb1[None, k * HS:(k + 1) * HS]),
            "w2es": to_bf(eW2[k * HS:(k + 1) * HS, :]),
            "b2e": np.ascontiguousarray(eb2[None, :]),
            "w2Ts": np.ascontiguousarray(
                (dW2T[k * DS:(k + 1) * DS, :] * np.float32(SCW)).astype(f8)),
            "w2s": np.ascontiguousarray(
                (dW2[:, k * DS:(k + 1) * DS] * np.float32(SCW)).astype(f8)),
            "w1": w1_bf,
            "w1Td": w1Td_bf,
            "w1T8d": w1T8,
            "b1d": b1d_bf,
            "sigw": sigv,
            "sel8": sel,
            "epsin": np.ascontiguousarray(eps[k * BL:(k + 1) * BL, :]),
        })
    return maps


_NC_CACHE = None


def kernel(**inputs) -> np.ndarray:
    global _NC_CACHE
    from concourse.bass_utils import run_bass_kernel_spmd
    if _NC_CACHE is None:
        _NC_CACHE = build_nc()
    nc = _NC_CACHE
    maps = shard_inputs(inputs)
    res = run_bass_kernel_spmd(nc, maps, list(range(NCORES)))
    outs = [res.results[k]["out"].reshape(BL) for k in range(NCORES)]
    return np.concatenate(outs).astype(np.float32)
